# revision 23
# baseline (speedup 1.0000x reference)
"""Trainium2 Bass kernel for nn_NormalizedDelinear (whitened linear layer).

Math (reference):
    X = x.reshape(-1, 512); N = X.shape[0]
    mean = X.mean(0);  cov = eps*I + (X-mean)^T (X-mean) / N
    C = newton_schulz_isqrt(cov, 5)
    w = weight.reshape(-1, 512) @ C;  b = bias - (w @ mean).reshape(1024, 2).sum(1)
    out = x @ w.reshape(1024, 1024).T + b

Distribution: data-parallel over the 65536 rows of x across 8 NeuronCores.
Each core computes partial S = X_loc^T X_loc and column sums s; the partials
are combined with an fp16 AllGather + local accumulate-DMA reduction (an
ncfw AllReduce costs ~6 x 10us of stepping latency; AllGather is one phase).
Every core then runs the replicated Newton-Schulz and weight transform and
computes its slice of the output GEMM.

Host-side staging (sharding prep): x is pre-cast to bf16 per core (halves
pass-A HBM traffic; pass A computes in bf16 anyway) and the weight is
staged pre-transposed as bf16 (kills the DRAM->DRAM cast + 16 xbar
transposes of the weight).

Queue assignment: x chunks + W^T loads + output stores on sync HWDGE; the
AllGather pack + x^T xbar transposes on scalar HWDGE (so the pack chains
straight after the PSUM->SBUF assembly on ACT); the collective trigger +
gather-accumulate DMAs on gpsimd SWDGE.
"""
import numpy as np

import concourse.bacc as bacc
import concourse.mybir as mybir
import concourse.tile as tile
import concourse.bass_utils as bass_utils

N_CORES = 8
ROWS = 65536
D = 1024
BLOCK = 512
EPS = 1e-5
N_ITER = 5
PART = 128
ROWS_PER_CORE = ROWS // N_CORES  # 8192
N_ROW_TILES = ROWS_PER_CORE // PART  # 64

f32 = mybir.dt.float32
f16 = mybir.dt.float16
bf16 = mybir.dt.bfloat16
ADD = mybir.AluOpType.add
MUL = mybir.AluOpType.mult
BYPASS = mybir.AluOpType.bypass

# pass-A chunk schedule (row tiles per DMA): small first chunks so the
# first matmul starts early, then 2-tile (512 KB bf16) chunks.
CHUNK_SIZES = [1, 1] + [2] * 31
assert sum(CHUNK_SIZES) == N_ROW_TILES
# pass-D step schedule: (chunk, tile offset, ntiles), aligned to chunks.
PASSD_STEPS = []
for _c, _sz in enumerate(CHUNK_SIZES):
    _off = 0
    while _off < _sz:
        _n = 1 if _c < 3 else min(2, _sz - _off)
        PASSD_STEPS.append((_c, _off, _n))
        _off += _n
PRE_T = 4  # x^T transposes pre-issued before the NS section (pd0 pool)

# AllGather pack layout: upper-triangle S blocks (widths 512,384,256,128)
# then the column sums pre-reduced to 512 values spread as [128, 4]
# (s[b*128+p] at [p, 1280+b]); fp16. The AllGather stages are
# byte-proportional (fold_n=2 x ~31 GB/s), so the payload is kept minimal.
UT_OFF = [0, BLOCK, BLOCK + 384, BLOCK + 384 + 256]
S_OFF = UT_OFF[3] + PART  # 1280
AR_W = S_OFF + 4  # 1284

WARM1 = 440  # N=256 PE keep-warm matmuls covering the AllGather wait
WARM2 = 80   # covering the gather-reduce window


def build_nc():
    nc = bacc.Bacc(
        "TRN2", target_bir_lowering=False, debug=False, num_devices=N_CORES
    )
    rows_pc = N_ROW_TILES * PART
    n_total = rows_pc * N_CORES * (D // BLOCK)  # global sample count N

    x_bf = nc.dram_tensor("x_bf", [rows_pc, D], bf16, kind="ExternalInput")
    weightT_bf = nc.dram_tensor("weightT_bf", [D, D], bf16, kind="ExternalInput")
    bias_rep = nc.dram_tensor("bias_rep", [PART, D], f32, kind="ExternalInput")
    # single diagonal block: eye15[p, f] = 1.5 * (p == f)
    eye15 = nc.dram_tensor("eye15", [PART, PART], bf16, kind="ExternalInput")
    id_f32 = nc.dram_tensor("id_f32", [PART, PART], f32, kind="ExternalInput")
    out = nc.dram_tensor("out", [rows_pc, D], f32, kind="ExternalOutput")

    with tile.TileContext(nc) as tc:
        _kernel_body(
            nc, tc, x_bf, weightT_bf, bias_rep, eye15, id_f32, out, n_total
        )
    nc.compile()
    return nc


def _kernel_body(nc, tc, x_bf, weightT_bf, bias_rep, eye15, id_f32, out, n_total):
    inv_n = 1.0 / float(n_total)
    n_chunks = len(CHUNK_SIZES)

    # ------------- long-lived pools (left side) + DRAM -------------------
    persist = tc.alloc_tile_pool(name="persist", bufs=1, side="left")
    consts = tc.alloc_tile_pool(name="consts", bufs=1, side="left")
    dram = tc.alloc_tile_pool(name="dram", bufs=1, space="DRAM")

    # resident bf16 x shard, chunked for dep granularity
    slab = [
        persist.tile([PART, sz, D], bf16, tag=f"slab{c}", name=f"slab{c}")
        for c, sz in enumerate(CHUNK_SIZES)
    ]
    b_rep = consts.tile([PART, D], f32, tag="b_rep")  # b' replicated later
    eye15_sb = consts.tile([PART, PART], bf16, tag="eye15")
    id_f = consts.tile([PART, PART], f32, tag="id_f")
    ones_f = consts.tile([PART, 1], f32, tag="ones_f")
    ones_row = consts.tile([1, PART], f32, tag="ones_row")
    ones_bf = consts.tile([PART, PART], bf16, tag="ones_bf")

    # x chunk loads FIRST on the SWDGE queue: nothing may delay chunk 0
    row0 = 0
    for c, sz in enumerate(CHUNK_SIZES):
        src = x_bf[row0 * PART:(row0 + sz) * PART, :].rearrange(
            "(t p) f -> p t f", p=PART
        )
        nc.gpsimd.dma_start(slab[c][:], src)
        row0 += sz

    nc.vector.memset(ones_f[:], 1.0)
    nc.vector.memset(ones_row[:], 1.0)
    nc.vector.memset(ones_bf[:], 1.0)

    # ------------- pass A: S = X^T X (upper triangle) + col sums ---------
    pa = tc.alloc_tile_pool(name="passA", bufs=1, side="right")
    pd0 = tc.alloc_tile_pool(name="passD0", bufs=PRE_T, side="right")
    acc = pa.tile([PART, BLOCK], bf16, tag="acc")  # running column sums
    nc.vector.memset(acc[:], 0.0)

    ps_S = tc.alloc_tile_pool(name="psumS", bufs=1, space="PSUM", side="right")
    # upper-triangle blocks of S packed into 3 banks: row 0 gets a full
    # bank; rows 1 (384) + 3 (128) share one; row 2 (256) gets one.
    sp0 = ps_S.tile([PART, BLOCK], f32, tag="S0", name="sp0")
    sp13 = ps_S.tile([PART, BLOCK], f32, tag="S13", name="sp13")
    sp2 = ps_S.tile([PART, 256], f32, tag="S2", name="sp2")
    s_psum = [sp0[:], sp13[:, 0:384], sp2[:], sp13[:, 384:BLOCK]]

    first = True
    for c, sz in enumerate(CHUNK_SIZES):
        for t in range(sz):
            for h in range(2):
                xt = slab[c][:, t, h * BLOCK:(h + 1) * BLOCK]  # [128, 512] bf16
                last = (c == n_chunks - 1 and t == sz - 1 and h == 1)
                for m in range(4):
                    nc.tensor.matmul(
                        s_psum[m][:],
                        xt[:, m * PART:(m + 1) * PART],
                        xt[:, m * PART:],
                        start=first,
                        stop=last,
                    )
                # column-sum accumulator on DVE (f32 += bf16)
                nc.vector.tensor_add(acc[:], acc[:], xt)
                first = False

    # ------------- pack the AllGather buffer (fp16, on ACT) ---------------
    early = tc.alloc_tile_pool(name="early", bufs=1, side="left")
    late = tc.alloc_tile_pool(name="late", bufs=1, side="right")
    ps_asm = tc.alloc_tile_pool(name="psumA", bufs=2, space="PSUM", side="left")

    packed = early.tile([PART, AR_W], f16, tag="packed")
    s_sb = late.tile([1, BLOCK], f32, tag="s_sb")
    with tc.high_priority():
        # local column sums reduced across partitions: s = ones^T @ acc
        scol = ps_asm.tile([PART, BLOCK], f32, tag="t", name="scol")
        nc.tensor.matmul(scol[0:1, :], ones_bf[:, 0:1], acc[:])
        nc.vector.tensor_copy(s_sb[:], scol[0:1, :])
        # upper-triangle blocks scaled by 1/N, straight from PSUM on ACT
        for m in range(4):
            nc.scalar.mul(packed[:, UT_OFF[m]:UT_OFF[m] + BLOCK - m * PART],
                          s_psum[m], inv_n)

        ag_in = dram.tile([PART, AR_W], f16, tag="ag_in")
        ag_out = dram.tile([N_CORES, PART, AR_W], f16, tag="ag_out")
        nc.scalar.dma_start(ag_in[:, 0:S_OFF], packed[:, 0:S_OFF])
        # s spread as [128, 4]: ag_in[p, S_OFF+b] = s[b*128+p]
        nc.gpsimd.dma_start(
            ag_in[:, S_OFF:].rearrange("p b -> b p"), s_sb[0:1, :]
        )
        nc.gpsimd.collective_compute(
            "AllGather",
            BYPASS,
            replica_groups=[list(range(N_CORES))],
            ins=[ag_in.opt()],
            outs=[ag_out.opt()],
        )
    ps_S.release()

    # consts + W^T loads on the SWDGE queue behind the trigger: they run
    # during the AllGather wait, never ahead of the pass-A x loads.
    nc.gpsimd.dma_start(eye15_sb[:], eye15[:])
    nc.gpsimd.dma_start(id_f[:], id_f32[:])
    nc.gpsimd.dma_start(b_rep[:], bias_rep[:])
    wth = tc.alloc_tile_pool(name="wth", bufs=1, side="right")
    WThs = []
    for j in range(2):
        WTh = wth.tile([PART, 4, D], bf16, tag="WTh", name="WTh")
        nc.gpsimd.dma_start(
            WTh[:],
            weightT_bf[j * BLOCK:(j + 1) * BLOCK, :].rearrange(
                "(d p) o -> p d o", p=PART
            ),
        )
        WThs.append(WTh)

    # x^T xbar transposes for the first two pass-D steps, on the scalar
    # queue right behind the pack so they run during the AllGather wait.
    def issue_xT(step, pool):
        c, off, nt = PASSD_STEPS[step]
        xT16 = pool.tile([PART, nt * 8, PART], bf16, tag="xT", name="xT")
        nc.scalar.dma_start(
            xT16[:],
            slab[c][:, off:off + nt, :].rearrange("p t f -> p (t f)"),
            transpose=True,
        )
        return xT16

    xT_pre = [issue_xT(s, pd0) for s in range(PRE_T)]

    # PE keep-warm through the AllGather wait (harmless matmuls, consumed
    # once far below). The source tile is written on DVE only after the
    # last column-sum add, so the scheduler cannot interleave these into
    # pass A ahead of the final S matmuls.
    warm_src = late.tile([PART, 256], bf16, tag="warm_src")
    nc.vector.tensor_copy(warm_src[:], acc[:, 0:256])
    warm = ps_asm.tile([PART, 256], f32, tag="warm", name="warm")
    for k in range(WARM1):
        nc.tensor.matmul(
            warm[:], warm_src[:, 0:PART], warm_src[:],
            start=True, stop=True,
        )

    # ------------- gather-reduce: 8 parallel loads + DVE in-place tree ----
    gpool = tc.alloc_tile_pool(name="gatherp", bufs=1, side="left")
    gather = gpool.tile([PART, N_CORES, AR_W], f16, tag="gather")
    for r in range(N_CORES):
        nc.sync.dma_start(gather[:, r, :], ag_out[r])
    for j in range(4):
        nc.vector.tensor_add(gather[:, j, :], gather[:, j, :], gather[:, j + 4, :])
    for j in range(2):
        nc.vector.tensor_add(gather[:, j, :], gather[:, j, :], gather[:, j + 2, :])

    S_sb = early.tile([PART, 4, BLOCK], f32, tag="S_sb")  # becomes A below
    scratch = early.tile([PART, BLOCK], f32, tag="scratch")
    for m in range(4):
        w = BLOCK - m * PART
        nc.vector.tensor_add(
            S_sb[:, m, m * PART:],
            gather[:, 0, UT_OFF[m]:UT_OFF[m] + w],
            gather[:, 1, UT_OFF[m]:UT_OFF[m] + w],
        )
    # global column sums, partition-spread: mean_sb[p, b] = mean[b*128+p]
    mean_sb = late.tile([PART, 4], f32, tag="mean_sb")
    mean4 = late.tile([PART, 4], f32, tag="mean4")
    nc.vector.tensor_add(
        mean4[:], gather[:, 0, S_OFF:], gather[:, 1, S_OFF:]
    )
    nc.vector.tensor_scalar_mul(mean_sb[:], mean4[:], inv_n)
    gpool.release()

    # warm2 reads the merged S row so it cannot start before the reduce.
    warm_src2 = late.tile([PART, 256], bf16, tag="warm_src2")
    nc.vector.tensor_copy(warm_src2[:], S_sb[:, 0, 0:256])
    warm2 = ps_asm.tile([PART, 256], f32, tag="warm", name="warm2")
    for k in range(WARM2):
        nc.tensor.matmul(
            warm2[:], warm_src2[:, 0:PART], warm_src2[:],
            start=True, stop=True,
        )

    # mirror the lower triangle: block (m, b) with b < m = (b, m)^T
    for m in range(4):
        for b in range(m):
            tp = ps_asm.tile([PART, BLOCK], f32, tag="t", name="tp")
            nc.tensor.transpose(
                tp[:, 0:PART], S_sb[:, b, m * PART:(m + 1) * PART], id_f[:]
            )
            nc.vector.tensor_copy(S_sb[:, m, b * PART:(b + 1) * PART], tp[:, 0:PART])

    # s_row[0, c] = sum-col c: PE transposes of the mean4 blocks
    sr_ps = ps_asm.tile([PART, BLOCK], f32, tag="t", name="sr")
    for b in range(4):
        nc.tensor.transpose(
            sr_ps[0:1, b * PART:(b + 1) * PART], mean4[:, b:b + 1], id_f[:]
        )
    s_row = late.tile([1, BLOCK], f32, tag="s_row")
    nc.vector.tensor_copy(s_row[:], sr_ps[0:1, :])

    # meanrow[p, c] = mean[c] (replicated down partitions), via PE ones
    mr_ps = ps_asm.tile([PART, BLOCK], f32, tag="t")
    nc.tensor.matmul(mr_ps[:], ones_row[:], s_row[:])
    meanrow = early.tile([PART, BLOCK], f32, tag="meanrow")
    nc.vector.tensor_scalar_mul(meanrow[:], mr_ps[:], inv_n)

    # A = S - mean mean^T + eps I (in place over S_sb); Frobenius row sums
    # of squares on ACT off the DVE critical path.
    A = S_sb
    eps_stripe = late.tile([PART, PART], f32, tag="eps_stripe")
    nc.vector.tensor_scalar_mul(eps_stripe[:], eye15_sb[:], EPS / 1.5)
    rowsq4 = early.tile([PART, 4], f32, tag="rowsq4")
    sqd = early.tile([PART, BLOCK], f32, tag="sqd")
    for b in range(4):
        nc.vector.tensor_scalar(
            scratch[:], meanrow[:], mean_sb[:, b:b + 1], None, op0=MUL
        )
        nc.vector.tensor_sub(A[:, b, :], A[:, b, :], scratch[:])
        d0 = b * PART
        nc.vector.tensor_add(
            A[:, b, d0:d0 + PART], A[:, b, d0:d0 + PART], eps_stripe[:]
        )
        nc.scalar.activation(
            sqd[:], A[:, b, :], mybir.ActivationFunctionType.Square,
            accum_out=rowsq4[:, b:b + 1],
        )

    # ------------- Frobenius norm; r = 1/||A||, q = 1/sqrt(||A||) ---------
    rowsq = early.tile([PART, 1], f32, tag="rowsq")
    nc.vector.tensor_reduce(rowsq[:], rowsq4[:], mybir.AxisListType.X, ADD)
    n2_ps = ps_asm.tile([PART, BLOCK], f32, tag="t")
    nc.tensor.matmul(n2_ps[0:1, 0:1], ones_f[:], rowsq[:])
    n2_sb = early.tile([1, 1], f32, tag="n2sb")
    nc.vector.tensor_copy(n2_sb[:], n2_ps[0:1, 0:1])
    # broadcast ||A||^2 to [128, 1] then compute per-partition scalars
    n2_bc = ps_asm.tile([PART, BLOCK], f32, tag="t")
    nc.tensor.matmul(n2_bc[:, 0:1], ones_row[:], n2_sb[:])
    nc.vector.tensor_copy(scratch[0:1, 0:1], warm[0:1, 0:1])
    nc.vector.tensor_copy(scratch[0:1, 1:2], warm2[0:1, 0:1])
    rq = late.tile([PART, 2], f32, tag="rq")
    nc.vector.reciprocal(rq[:, 0:1], n2_bc[:, 0:1])    # 1/||A||^2
    nc.scalar.sqrt(rq[:, 0:1], rq[:, 0:1])             # r = 1/||A||
    nc.scalar.sqrt(rq[:, 1:2], rq[:, 0:1])             # q = 1/sqrt(||A||)

    ps_asm.release()

    # ------------- Newton-Schulz (bf16 matmuls, fp32 PSUM) ----------------
    ns = tc.alloc_tile_pool(name="ns", bufs=1, side="right")
    ps_ns = tc.alloc_tile_pool(name="psumNS", bufs=8, space="PSUM", side="left")

    Yb = [
        [ns.tile([PART, BLOCK], bf16, tag=f"Y{i}{b}", name=f"Y{i}{b}")
         for b in range(4)]
        for i in range(2)
    ]
    Zb = [
        [ns.tile([PART, BLOCK], bf16, tag=f"Z{i}{b}", name=f"Z{i}{b}")
         for b in range(4)]
        for i in range(2)
    ]
    T = [ns.tile([PART, BLOCK], bf16, tag=f"T{b}", name=f"T{b}") for b in range(4)]
    C = [wth.tile([PART, BLOCK], bf16, tag=f"C{b}", name=f"C{b}") for b in range(4)]
    rep = [
        wth.tile([PART, PART], bf16, tag=f"rep{b}", name=f"rep{b}")
        for b in range(4)
    ]

    for b in range(4):  # Y0 = A * r
        nc.vector.tensor_scalar(Yb[0][b][:], A[:, b, :], rq[:, 0:1], None, op0=MUL)

    early.release()
    wts2 = tc.alloc_tile_pool(name="wts2", bufs=1, side="left")

    def mm512(dst, L, R, copy_engine, scale=None):
        """dst = L(stored)^T @ R for 512x512 bf16 operands as 4 [128, 512] tiles.

        Valid when L is symmetric (or its transpose is wanted). dst must not
        alias L or R. copy_engine: 'v' DVE / 's' ACT for the psum->sbuf copy.
        """
        for mb in range(4):
            pt = ps_ns.tile([PART, BLOCK], f32, tag="mm", name="mm")
            for kb in range(4):
                nc.tensor.matmul(
                    pt[:],
                    L[kb][:, mb * PART:(mb + 1) * PART],
                    R[kb][:],
                    start=(kb == 0),
                    stop=(kb == 3),
                )
            if scale is not None:
                nc.vector.tensor_scalar(dst[mb][:], pt[:], scale, None, op0=MUL)
            elif copy_engine == "v":
                nc.vector.tensor_copy(dst[mb][:], pt[:])
            else:
                nc.scalar.copy(dst[mb][:], pt[:])

    def build_T(p_blocks):
        # T = 1.5 I - 0.5 P: full-width scale on ACT, then the diagonal
        # 128-wide 1.5*I add on DVE (eye15 block b is zero off that stripe).
        for b in range(4):
            nc.scalar.mul(T[b][:], p_blocks[b], -0.5)
            d0 = b * PART
            nc.vector.tensor_add(
                T[b][:, d0:d0 + PART], T[b][:, d0:d0 + PART], eye15_sb[:]
            )

    # iter 1: Z0 = I, so P = Y0; T1 = 1.5I - 0.5 Y0; Y1 = Y0 @ T1; Z1 = T1
    Y, Z = Yb[0], Zb[0]
    build_T([Y[b][:] for b in range(4)])
    mm512(Yb[1], Y, T, "s")  # Y1 = Y0 @ T1  (Y0 symmetric)
    for b in range(4):
        nc.scalar.copy(Zb[1][b][:], T[b][:])
    Y, Z = Yb[1], Zb[1]

    for it in range(1, N_ITER):
        # P = Z @ Y -> psum tiles; T = 1.5I - 0.5P
        pt_blocks = []
        for mb in range(4):
            pt = ps_ns.tile([PART, BLOCK], f32, tag="mm", name="mm")
            for kb in range(4):
                nc.tensor.matmul(
                    pt[:],
                    Z[kb][:, mb * PART:(mb + 1) * PART],
                    Y[kb][:],
                    start=(kb == 0),
                    stop=(kb == 3),
                )
            pt_blocks.append(pt)
        build_T([pt[:] for pt in pt_blocks])
        if it < N_ITER - 1:
            Yn, Zn = Yb[(it + 1) % 2], Zb[(it + 1) % 2]
            # Y_next = Y @ T and Z_next = T @ Z (T symmetric), interleaved
            # block-by-block so each psum->sbuf copy hides under the other
            # product's matmul group.
            for mb in range(4):
                pty = ps_ns.tile([PART, BLOCK], f32, tag="mm", name="mm")
                for kb in range(4):
                    nc.tensor.matmul(
                        pty[:],
                        Y[kb][:, mb * PART:(mb + 1) * PART],
                        T[kb][:],
                        start=(kb == 0),
                        stop=(kb == 3),
                    )
                ptz = ps_ns.tile([PART, BLOCK], f32, tag="mm", name="mm")
                for kb in range(4):
                    nc.tensor.matmul(
                        ptz[:],
                        T[kb][:, mb * PART:(mb + 1) * PART],
                        Z[kb][:],
                        start=(kb == 0),
                        stop=(kb == 3),
                    )
                nc.scalar.copy(Yn[mb][:], pty[:])
                nc.vector.tensor_copy(Zn[mb][:], ptz[:])
            Y, Z = Yn, Zn
        else:
            # final iteration: only Z needed; C = q * (T @ Z).
            mm512(C, T, Z, "v", scale=rq[:, 1:2])

    # mean replicated blocks: rep_b[p, f] = mean[b*128+p]
    for b in range(4):
        nc.vector.tensor_scalar(
            rep[b][:], ones_bf[:], mean_sb[:, b:b + 1], None, op0=MUL
        )
    ns.release()

    # ------------- wT = C^T @ W^T ; b' = bias - pair-summed w @ mean -------
    wT = wts2.tile([PART, 8, D], bf16, tag="wT")  # w_full^T[i, o]
    for j in range(2):
        WTh = WThs[j]
        for cb in range(4):
            for nb in range(2):
                pt = ps_ns.tile([PART, BLOCK], f32, tag="mm", name="mm")
                for db in range(4):
                    nc.tensor.matmul(
                        pt[:],
                        C[db][:, cb * PART:(cb + 1) * PART],
                        WTh[:, db, nb * BLOCK:(nb + 1) * BLOCK],
                        start=(db == 0),
                        stop=(db == 3),
                    )
                nc.scalar.copy(
                    wT[:, j * 4 + cb, nb * BLOCK:(nb + 1) * BLOCK], pt[:]
                )

    bc_ps = [
        ps_ns.tile([PART, BLOCK], f32, tag="mm", name=f"bc{i}") for i in range(2)
    ]
    for nb in range(2):
        for g in range(8):
            nc.tensor.matmul(
                bc_ps[nb][:],
                rep[g % 4][:],
                wT[:, g, nb * BLOCK:(nb + 1) * BLOCK],
                start=(g == 0),
                stop=(g == 7),
            )
    for nb in range(2):
        nc.vector.tensor_sub(
            b_rep[:, nb * BLOCK:(nb + 1) * BLOCK],
            b_rep[:, nb * BLOCK:(nb + 1) * BLOCK],
            bc_ps[nb][:],
        )

    wth.release()
    late.release()

    # ------------- pass D: out = x @ w^T + b' -----------------------------
    pd = tc.alloc_tile_pool(name="passD", bufs=6, side="right")
    pd_out = tc.alloc_tile_pool(name="passDout", bufs=3, side="right")

    n_steps = len(PASSD_STEPS)
    for st in range(n_steps):
        c, off, nt = PASSD_STEPS[st]
        xT16 = xT_pre[st] if st < PRE_T else issue_xT(st, pd)
        ot = pd_out.tile([PART, nt, D], f32, tag="ot", name="ot")
        for tt in range(nt):
            pts = [ps_ns.tile([PART, BLOCK], f32, tag="mm", name=f"outp{nb}")
                   for nb in range(2)]
            for g in range(8):
                for nb in range(2):
                    nc.tensor.matmul(
                        pts[nb][:],
                        xT16[:, tt * 8 + g, :],
                        wT[:, g, nb * BLOCK:(nb + 1) * BLOCK],
                        start=(g == 0),
                        stop=(g == 7),
                    )
            for nb in range(2):
                nc.vector.tensor_add(
                    ot[:, tt, nb * BLOCK:(nb + 1) * BLOCK], pts[nb][:],
                    b_rep[:, nb * BLOCK:(nb + 1) * BLOCK],
                )
        r0 = sum(CHUNK_SIZES[:c]) + off
        nc.sync.dma_start(
            out[r0 * PART:(r0 + nt) * PART, :].rearrange(
                "(t p) f -> p t f", p=PART
            ),
            ot[:],
        )

    ps_ns.release()
    pd_out.release()
    pd.release()
    pd0.release()
    pa.release()
    wts2.release()
    consts.release()
    persist.release()
    dram.release()


# ---------------------------------------------------------------------------
def make_aux_inputs():
    import ml_dtypes

    return {
        "eye15": (1.5 * np.eye(PART)).astype(ml_dtypes.bfloat16),
        "id_f32": np.eye(PART, dtype=np.float32),
    }


_NC_CACHE = {}


def get_nc():
    if "nc" not in _NC_CACHE:
        _NC_CACHE["nc"] = build_nc()
    return _NC_CACHE["nc"]


def make_in_maps(x, weight, bias):
    import ml_dtypes

    aux = make_aux_inputs()
    x_bf = np.ascontiguousarray(
        np.asarray(x, dtype=np.float32).astype(ml_dtypes.bfloat16)
    )
    weightT_bf = np.ascontiguousarray(
        np.asarray(weight, dtype=np.float32).T.astype(ml_dtypes.bfloat16)
    )
    bias = np.asarray(bias, dtype=np.float32)
    bias_rep = np.ascontiguousarray(np.tile(bias[None, :], (PART, 1)))
    rows_pc = N_ROW_TILES * PART
    in_maps = []
    for i in range(N_CORES):
        m = {"x_bf": x_bf[i * rows_pc:(i + 1) * rows_pc],
             "weightT_bf": weightT_bf, "bias_rep": bias_rep}
        m.update(aux)
        in_maps.append(m)
    return in_maps


def kernel(x, weight, bias):
    nc = get_nc()
    in_maps = make_in_maps(x, weight, bias)
    res = bass_utils.run_bass_kernel_spmd(
        nc, in_maps, core_ids=list(range(N_CORES))
    )
    return np.concatenate([r["out"] for r in res.results], axis=0)


# revision 24
# speedup vs baseline: 1.0274x; 1.0274x over previous
"""Trainium2 Bass kernel for nn_NormalizedDelinear (whitened linear layer).

Math (reference):
    X = x.reshape(-1, 512); N = X.shape[0]
    mean = X.mean(0);  cov = eps*I + (X-mean)^T (X-mean) / N
    C = newton_schulz_isqrt(cov, 5)
    w = weight.reshape(-1, 512) @ C;  b = bias - (w @ mean).reshape(1024, 2).sum(1)
    out = x @ w.reshape(1024, 1024).T + b

Distribution: data-parallel over the 65536 rows of x across 8 NeuronCores.
Each core computes partial S = X_loc^T X_loc and column sums s; the partials
are combined with an fp16 AllGather + local accumulate-DMA reduction (an
ncfw AllReduce costs ~6 x 10us of stepping latency; AllGather is one phase).
Every core then runs the replicated Newton-Schulz and weight transform and
computes its slice of the output GEMM.

Host-side staging (sharding prep): x is pre-cast to bf16 per core (halves
pass-A HBM traffic; pass A computes in bf16 anyway) and the weight is
staged pre-transposed as bf16 (kills the DRAM->DRAM cast + 16 xbar
transposes of the weight).

Queue assignment: x chunks + W^T loads + output stores on sync HWDGE; the
AllGather pack + x^T xbar transposes on scalar HWDGE (so the pack chains
straight after the PSUM->SBUF assembly on ACT); the collective trigger +
gather-accumulate DMAs on gpsimd SWDGE.
"""
import numpy as np

import concourse.bacc as bacc
import concourse.mybir as mybir
import concourse.tile as tile
import concourse.bass_utils as bass_utils

N_CORES = 8
ROWS = 65536
D = 1024
BLOCK = 512
EPS = 1e-5
N_ITER = 5
PART = 128
ROWS_PER_CORE = ROWS // N_CORES  # 8192
N_ROW_TILES = ROWS_PER_CORE // PART  # 64

f32 = mybir.dt.float32
f16 = mybir.dt.float16
bf16 = mybir.dt.bfloat16
ADD = mybir.AluOpType.add
MUL = mybir.AluOpType.mult
BYPASS = mybir.AluOpType.bypass

# pass-A chunk schedule (row tiles per DMA): small first chunks so the
# first matmul starts early, then 2-tile (512 KB bf16) chunks.
CHUNK_SIZES = [1, 1] + [2] * 31
assert sum(CHUNK_SIZES) == N_ROW_TILES
# pass-D step schedule: (chunk, tile offset, ntiles), aligned to chunks.
PASSD_STEPS = []
for _c, _sz in enumerate(CHUNK_SIZES):
    _off = 0
    while _off < _sz:
        _n = 1 if _c < 3 else min(2, _sz - _off)
        PASSD_STEPS.append((_c, _off, _n))
        _off += _n
PRE_T = 4  # x^T transposes pre-issued before the NS section (pd0 pool)

# AllGather pack layout: upper-triangle S blocks (widths 512,384,256,128)
# then the column sums pre-reduced to 512 values spread as [128, 4]
# (s[b*128+p] at [p, 1280+b]); fp16. The AllGather stages are
# byte-proportional (fold_n=2 x ~31 GB/s), so the payload is kept minimal.
UT_OFF = [0, BLOCK, BLOCK + 384, BLOCK + 384 + 256]
S_OFF = UT_OFF[3] + PART  # 1280
AR_W = S_OFF + 4  # 1284

WARM1 = 520  # N=256 PE keep-warm matmuls covering the AllGather wait
WARM2 = 40   # covering the gather-reduce window


def build_nc():
    nc = bacc.Bacc(
        "TRN2", target_bir_lowering=False, debug=False, num_devices=N_CORES
    )
    rows_pc = N_ROW_TILES * PART
    n_total = rows_pc * N_CORES * (D // BLOCK)  # global sample count N

    x_bf = nc.dram_tensor("x_bf", [rows_pc, D], bf16, kind="ExternalInput")
    weightT_bf = nc.dram_tensor("weightT_bf", [D, D], bf16, kind="ExternalInput")
    bias_rep = nc.dram_tensor("bias_rep", [PART, D], f32, kind="ExternalInput")
    # single diagonal block: eye15[p, f] = 1.5 * (p == f)
    eye15 = nc.dram_tensor("eye15", [PART, PART], bf16, kind="ExternalInput")
    id_f32 = nc.dram_tensor("id_f32", [PART, PART], f32, kind="ExternalInput")
    out = nc.dram_tensor("out", [rows_pc, D], f32, kind="ExternalOutput")

    with tile.TileContext(nc) as tc:
        _kernel_body(
            nc, tc, x_bf, weightT_bf, bias_rep, eye15, id_f32, out, n_total
        )
    nc.compile()
    return nc


def _kernel_body(nc, tc, x_bf, weightT_bf, bias_rep, eye15, id_f32, out, n_total):
    inv_n = 1.0 / float(n_total)
    n_chunks = len(CHUNK_SIZES)

    # ------------- long-lived pools (left side) + DRAM -------------------
    persist = tc.alloc_tile_pool(name="persist", bufs=1, side="left")
    consts = tc.alloc_tile_pool(name="consts", bufs=1, side="left")
    dram = tc.alloc_tile_pool(name="dram", bufs=1, space="DRAM")

    # resident bf16 x shard, chunked for dep granularity
    slab = [
        persist.tile([PART, sz, D], bf16, tag=f"slab{c}", name=f"slab{c}")
        for c, sz in enumerate(CHUNK_SIZES)
    ]
    b_rep = consts.tile([PART, D], f32, tag="b_rep")  # b' replicated later
    eye15_sb = consts.tile([PART, PART], bf16, tag="eye15")
    id_f = consts.tile([PART, PART], f32, tag="id_f")
    ones_f = consts.tile([PART, 1], f32, tag="ones_f")
    ones_row = consts.tile([1, PART], f32, tag="ones_row")
    ones_bf = consts.tile([PART, PART], bf16, tag="ones_bf")

    # x chunk loads FIRST on the SWDGE queue: nothing may delay chunk 0
    row0 = 0
    for c, sz in enumerate(CHUNK_SIZES):
        src = x_bf[row0 * PART:(row0 + sz) * PART, :].rearrange(
            "(t p) f -> p t f", p=PART
        )
        nc.gpsimd.dma_start(slab[c][:], src)
        row0 += sz

    nc.vector.memset(ones_f[:], 1.0)
    nc.vector.memset(ones_row[:], 1.0)
    nc.vector.memset(ones_bf[:], 1.0)

    # ------------- pass A: S = X^T X (upper triangle) + col sums ---------
    pa = tc.alloc_tile_pool(name="passA", bufs=1, side="right")
    pd0 = tc.alloc_tile_pool(name="passD0", bufs=PRE_T, side="right")
    acc = pa.tile([PART, BLOCK], bf16, tag="acc")  # running column sums
    nc.vector.memset(acc[:], 0.0)

    ps_S = tc.alloc_tile_pool(name="psumS", bufs=1, space="PSUM", side="right")
    # upper-triangle blocks of S packed into 3 banks: row 0 gets a full
    # bank; rows 1 (384) + 3 (128) share one; row 2 (256) gets one.
    sp0 = ps_S.tile([PART, BLOCK], f32, tag="S0", name="sp0")
    sp13 = ps_S.tile([PART, BLOCK], f32, tag="S13", name="sp13")
    sp2 = ps_S.tile([PART, 256], f32, tag="S2", name="sp2")
    s_psum = [sp0[:], sp13[:, 0:384], sp2[:], sp13[:, 384:BLOCK]]

    first = True
    for c, sz in enumerate(CHUNK_SIZES):
        for t in range(sz):
            for h in range(2):
                xt = slab[c][:, t, h * BLOCK:(h + 1) * BLOCK]  # [128, 512] bf16
                last = (c == n_chunks - 1 and t == sz - 1 and h == 1)
                for m in range(4):
                    nc.tensor.matmul(
                        s_psum[m][:],
                        xt[:, m * PART:(m + 1) * PART],
                        xt[:, m * PART:],
                        start=first,
                        stop=last,
                    )
                # column-sum accumulator on DVE (f32 += bf16)
                nc.vector.tensor_add(acc[:], acc[:], xt)
                first = False

    # ------------- pack the AllGather buffer (fp16, on ACT) ---------------
    early = tc.alloc_tile_pool(name="early", bufs=1, side="left")
    late = tc.alloc_tile_pool(name="late", bufs=1, side="right")
    ps_asm = tc.alloc_tile_pool(name="psumA", bufs=2, space="PSUM", side="left")

    packed = early.tile([PART, AR_W], f16, tag="packed")
    s_sb = late.tile([1, BLOCK], f32, tag="s_sb")
    with tc.high_priority():
        # local column sums reduced across partitions: s = ones^T @ acc
        scol = ps_asm.tile([PART, BLOCK], f32, tag="t", name="scol")
        nc.tensor.matmul(scol[0:1, :], ones_bf[:, 0:1], acc[:])
        nc.vector.tensor_copy(s_sb[:], scol[0:1, :])
        # upper-triangle blocks scaled by 1/N, straight from PSUM on ACT
        for m in range(4):
            nc.scalar.mul(packed[:, UT_OFF[m]:UT_OFF[m] + BLOCK - m * PART],
                          s_psum[m], inv_n)

        ag_in = dram.tile([PART, AR_W], f16, tag="ag_in")
        ag_out = dram.tile([N_CORES, PART, AR_W], f16, tag="ag_out")
        nc.scalar.dma_start(ag_in[:, 0:S_OFF], packed[:, 0:S_OFF])
        # s spread as [128, 4]: ag_in[p, S_OFF+b] = s[b*128+p]
        nc.gpsimd.dma_start(
            ag_in[:, S_OFF:].rearrange("p b -> b p"), s_sb[0:1, :]
        )
        nc.gpsimd.collective_compute(
            "AllGather",
            BYPASS,
            replica_groups=[list(range(N_CORES))],
            ins=[ag_in.opt()],
            outs=[ag_out.opt()],
        )
    ps_S.release()

    # consts + W^T loads on the SWDGE queue behind the trigger: they run
    # during the AllGather wait, never ahead of the pass-A x loads.
    nc.gpsimd.dma_start(eye15_sb[:], eye15[:])
    nc.gpsimd.dma_start(id_f[:], id_f32[:])
    nc.gpsimd.dma_start(b_rep[:], bias_rep[:])
    wth = tc.alloc_tile_pool(name="wth", bufs=1, side="right")
    WThs = []
    for j in range(2):
        WTh = wth.tile([PART, 4, D], bf16, tag="WTh", name="WTh")
        nc.gpsimd.dma_start(
            WTh[:],
            weightT_bf[j * BLOCK:(j + 1) * BLOCK, :].rearrange(
                "(d p) o -> p d o", p=PART
            ),
        )
        WThs.append(WTh)

    # x^T xbar transposes for the first two pass-D steps, on the scalar
    # queue right behind the pack so they run during the AllGather wait.
    def issue_xT(step, pool):
        c, off, nt = PASSD_STEPS[step]
        xT16 = pool.tile([PART, nt * 8, PART], bf16, tag="xT", name="xT")
        nc.scalar.dma_start(
            xT16[:],
            slab[c][:, off:off + nt, :].rearrange("p t f -> p (t f)"),
            transpose=True,
        )
        return xT16

    xT_pre = [issue_xT(s, pd0) for s in range(PRE_T)]

    # PE keep-warm through the AllGather wait (harmless matmuls, consumed
    # once far below). The source tile is written on DVE only after the
    # last column-sum add, so the scheduler cannot interleave these into
    # pass A ahead of the final S matmuls.
    warm_src = late.tile([PART, 256], bf16, tag="warm_src")
    nc.vector.tensor_copy(warm_src[:], acc[:, 0:256])
    warm = ps_asm.tile([PART, 256], f32, tag="warm", name="warm")
    for k in range(WARM1):
        nc.tensor.matmul(
            warm[:], warm_src[:, 0:PART], warm_src[:],
            start=True, stop=True,
        )

    # ------------- gather-reduce: 8 parallel loads + DVE in-place tree ----
    gpool = tc.alloc_tile_pool(name="gatherp", bufs=1, side="left")
    gather = gpool.tile([PART, N_CORES, AR_W], f16, tag="gather")
    for r in range(N_CORES):
        eng = nc.sync if r % 2 == 0 else nc.scalar
        eng.dma_start(gather[:, r, :], ag_out[r])
    for j in range(4):
        nc.vector.tensor_add(gather[:, j, :], gather[:, j, :], gather[:, j + 4, :])
    for j in range(2):
        nc.vector.tensor_add(gather[:, j, :], gather[:, j, :], gather[:, j + 2, :])

    S_sb = early.tile([PART, 4, BLOCK], f32, tag="S_sb")  # becomes A below
    scratch = early.tile([PART, BLOCK], f32, tag="scratch")
    for m in range(4):
        w = BLOCK - m * PART
        nc.vector.tensor_add(
            S_sb[:, m, m * PART:],
            gather[:, 0, UT_OFF[m]:UT_OFF[m] + w],
            gather[:, 1, UT_OFF[m]:UT_OFF[m] + w],
        )
    # global column sums, partition-spread: mean_sb[p, b] = mean[b*128+p]
    mean_sb = late.tile([PART, 4], f32, tag="mean_sb")
    mean4 = late.tile([PART, 4], f32, tag="mean4")
    nc.vector.tensor_add(
        mean4[:], gather[:, 0, S_OFF:], gather[:, 1, S_OFF:]
    )
    nc.vector.tensor_scalar_mul(mean_sb[:], mean4[:], inv_n)
    gpool.release()

    # warm2 reads the merged S row so it cannot start before the reduce.
    warm_src2 = late.tile([PART, 256], bf16, tag="warm_src2")
    nc.vector.tensor_copy(warm_src2[:], S_sb[:, 0, 0:256])
    warm2 = ps_asm.tile([PART, 256], f32, tag="warm", name="warm2")
    for k in range(WARM2):
        nc.tensor.matmul(
            warm2[:], warm_src2[:, 0:PART], warm_src2[:],
            start=True, stop=True,
        )

    # mirror the lower triangle: block (m, b) with b < m = (b, m)^T
    for m in range(4):
        for b in range(m):
            tp = ps_asm.tile([PART, BLOCK], f32, tag="t", name="tp")
            nc.tensor.transpose(
                tp[:, 0:PART], S_sb[:, b, m * PART:(m + 1) * PART], id_f[:]
            )
            nc.vector.tensor_copy(S_sb[:, m, b * PART:(b + 1) * PART], tp[:, 0:PART])

    # s_row[0, c] = sum-col c: PE transposes of the mean4 blocks
    sr_ps = ps_asm.tile([PART, BLOCK], f32, tag="t", name="sr")
    for b in range(4):
        nc.tensor.transpose(
            sr_ps[0:1, b * PART:(b + 1) * PART], mean4[:, b:b + 1], id_f[:]
        )
    s_row = late.tile([1, BLOCK], f32, tag="s_row")
    nc.vector.tensor_copy(s_row[:], sr_ps[0:1, :])

    # meanrow[p, c] = mean[c] (replicated down partitions), via PE ones
    mr_ps = ps_asm.tile([PART, BLOCK], f32, tag="t")
    nc.tensor.matmul(mr_ps[:], ones_row[:], s_row[:])
    meanrow = early.tile([PART, BLOCK], f32, tag="meanrow")
    nc.vector.tensor_scalar_mul(meanrow[:], mr_ps[:], inv_n)

    # A = S - mean mean^T + eps I (in place over S_sb); Frobenius row sums
    # of squares on ACT off the DVE critical path.
    A = S_sb
    eps_stripe = late.tile([PART, PART], f32, tag="eps_stripe")
    nc.vector.tensor_scalar_mul(eps_stripe[:], eye15_sb[:], EPS / 1.5)
    rowsq4 = early.tile([PART, 4], f32, tag="rowsq4")
    sqd = early.tile([PART, BLOCK], f32, tag="sqd")
    for b in range(4):
        nc.vector.tensor_scalar(
            scratch[:], meanrow[:], mean_sb[:, b:b + 1], None, op0=MUL
        )
        nc.vector.tensor_sub(A[:, b, :], A[:, b, :], scratch[:])
        d0 = b * PART
        nc.vector.tensor_add(
            A[:, b, d0:d0 + PART], A[:, b, d0:d0 + PART], eps_stripe[:]
        )
        nc.scalar.activation(
            sqd[:], A[:, b, :], mybir.ActivationFunctionType.Square,
            accum_out=rowsq4[:, b:b + 1],
        )

    # ------------- Frobenius norm; r = 1/||A||, q = 1/sqrt(||A||) ---------
    rowsq = early.tile([PART, 1], f32, tag="rowsq")
    nc.vector.tensor_reduce(rowsq[:], rowsq4[:], mybir.AxisListType.X, ADD)
    n2_ps = ps_asm.tile([PART, BLOCK], f32, tag="t")
    nc.tensor.matmul(n2_ps[0:1, 0:1], ones_f[:], rowsq[:])
    n2_sb = early.tile([1, 1], f32, tag="n2sb")
    nc.vector.tensor_copy(n2_sb[:], n2_ps[0:1, 0:1])
    # broadcast ||A||^2 to [128, 1] then compute per-partition scalars
    n2_bc = ps_asm.tile([PART, BLOCK], f32, tag="t")
    nc.tensor.matmul(n2_bc[:, 0:1], ones_row[:], n2_sb[:])
    nc.vector.tensor_copy(scratch[0:1, 0:1], warm[0:1, 0:1])
    nc.vector.tensor_copy(scratch[0:1, 1:2], warm2[0:1, 0:1])
    rq = late.tile([PART, 2], f32, tag="rq")
    nc.vector.reciprocal(rq[:, 0:1], n2_bc[:, 0:1])    # 1/||A||^2
    nc.scalar.sqrt(rq[:, 0:1], rq[:, 0:1])             # r = 1/||A||
    nc.scalar.sqrt(rq[:, 1:2], rq[:, 0:1])             # q = 1/sqrt(||A||)

    ps_asm.release()

    # ------------- Newton-Schulz (bf16 matmuls, fp32 PSUM) ----------------
    ns = tc.alloc_tile_pool(name="ns", bufs=1, side="right")
    ps_ns = tc.alloc_tile_pool(name="psumNS", bufs=8, space="PSUM", side="left")

    Yb = [
        [ns.tile([PART, BLOCK], bf16, tag=f"Y{i}{b}", name=f"Y{i}{b}")
         for b in range(4)]
        for i in range(2)
    ]
    Zb = [
        [ns.tile([PART, BLOCK], bf16, tag=f"Z{i}{b}", name=f"Z{i}{b}")
         for b in range(4)]
        for i in range(2)
    ]
    T = [ns.tile([PART, BLOCK], bf16, tag=f"T{b}", name=f"T{b}") for b in range(4)]
    C = [wth.tile([PART, BLOCK], bf16, tag=f"C{b}", name=f"C{b}") for b in range(4)]
    rep = [
        wth.tile([PART, PART], bf16, tag=f"rep{b}", name=f"rep{b}")
        for b in range(4)
    ]

    for b in range(4):  # Y0 = A * r
        nc.vector.tensor_scalar(Yb[0][b][:], A[:, b, :], rq[:, 0:1], None, op0=MUL)

    early.release()
    wts2 = tc.alloc_tile_pool(name="wts2", bufs=1, side="left")

    def mm512(dst, L, R, copy_engine, scale=None):
        """dst = L(stored)^T @ R for 512x512 bf16 operands as 4 [128, 512] tiles.

        Valid when L is symmetric (or its transpose is wanted). dst must not
        alias L or R. copy_engine: 'v' DVE / 's' ACT for the psum->sbuf copy.
        """
        for mb in range(4):
            pt = ps_ns.tile([PART, BLOCK], f32, tag="mm", name="mm")
            for kb in range(4):
                nc.tensor.matmul(
                    pt[:],
                    L[kb][:, mb * PART:(mb + 1) * PART],
                    R[kb][:],
                    start=(kb == 0),
                    stop=(kb == 3),
                )
            if scale is not None:
                nc.vector.tensor_scalar(dst[mb][:], pt[:], scale, None, op0=MUL)
            elif copy_engine == "v":
                nc.vector.tensor_copy(dst[mb][:], pt[:])
            else:
                nc.scalar.copy(dst[mb][:], pt[:])

    def build_T(p_blocks):
        # T = 1.5 I - 0.5 P: full-width scale on ACT, then the diagonal
        # 128-wide 1.5*I add on DVE (eye15 block b is zero off that stripe).
        for b in range(4):
            nc.scalar.mul(T[b][:], p_blocks[b], -0.5)
            d0 = b * PART
            nc.vector.tensor_add(
                T[b][:, d0:d0 + PART], T[b][:, d0:d0 + PART], eye15_sb[:]
            )

    # iter 1: Z0 = I, so P = Y0; T1 = 1.5I - 0.5 Y0; Y1 = Y0 @ T1; Z1 = T1
    Y, Z = Yb[0], Zb[0]
    build_T([Y[b][:] for b in range(4)])
    mm512(Yb[1], Y, T, "s")  # Y1 = Y0 @ T1  (Y0 symmetric)
    for b in range(4):
        nc.scalar.copy(Zb[1][b][:], T[b][:])
    Y, Z = Yb[1], Zb[1]

    for it in range(1, N_ITER):
        # P = Z @ Y -> psum tiles; T = 1.5I - 0.5P
        pt_blocks = []
        for mb in range(4):
            pt = ps_ns.tile([PART, BLOCK], f32, tag="mm", name="mm")
            for kb in range(4):
                nc.tensor.matmul(
                    pt[:],
                    Z[kb][:, mb * PART:(mb + 1) * PART],
                    Y[kb][:],
                    start=(kb == 0),
                    stop=(kb == 3),
                )
            pt_blocks.append(pt)
        build_T([pt[:] for pt in pt_blocks])
        if it < N_ITER - 1:
            Yn, Zn = Yb[(it + 1) % 2], Zb[(it + 1) % 2]
            # Y_next = Y @ T and Z_next = T @ Z (T symmetric), interleaved
            # block-by-block so each psum->sbuf copy hides under the other
            # product's matmul group.
            for mb in range(4):
                pty = ps_ns.tile([PART, BLOCK], f32, tag="mm", name="mm")
                for kb in range(4):
                    nc.tensor.matmul(
                        pty[:],
                        Y[kb][:, mb * PART:(mb + 1) * PART],
                        T[kb][:],
                        start=(kb == 0),
                        stop=(kb == 3),
                    )
                ptz = ps_ns.tile([PART, BLOCK], f32, tag="mm", name="mm")
                for kb in range(4):
                    nc.tensor.matmul(
                        ptz[:],
                        T[kb][:, mb * PART:(mb + 1) * PART],
                        Z[kb][:],
                        start=(kb == 0),
                        stop=(kb == 3),
                    )
                nc.scalar.copy(Yn[mb][:], pty[:])
                nc.vector.tensor_copy(Zn[mb][:], ptz[:])
            Y, Z = Yn, Zn
        else:
            # final iteration: only Z needed; C = q * (T @ Z).
            mm512(C, T, Z, "v", scale=rq[:, 1:2])

    # mean replicated blocks: rep_b[p, f] = mean[b*128+p]
    for b in range(4):
        nc.vector.tensor_scalar(
            rep[b][:], ones_bf[:], mean_sb[:, b:b + 1], None, op0=MUL
        )
    ns.release()

    # ------------- wT = C^T @ W^T ; b' = bias - pair-summed w @ mean -------
    wT = wts2.tile([PART, 8, D], bf16, tag="wT")  # w_full^T[i, o]
    for j in range(2):
        WTh = WThs[j]
        for cb in range(4):
            for nb in range(2):
                pt = ps_ns.tile([PART, BLOCK], f32, tag="mm", name="mm")
                for db in range(4):
                    nc.tensor.matmul(
                        pt[:],
                        C[db][:, cb * PART:(cb + 1) * PART],
                        WTh[:, db, nb * BLOCK:(nb + 1) * BLOCK],
                        start=(db == 0),
                        stop=(db == 3),
                    )
                nc.scalar.copy(
                    wT[:, j * 4 + cb, nb * BLOCK:(nb + 1) * BLOCK], pt[:]
                )

    bc_ps = [
        ps_ns.tile([PART, BLOCK], f32, tag="mm", name=f"bc{i}") for i in range(2)
    ]
    for nb in range(2):
        for g in range(8):
            nc.tensor.matmul(
                bc_ps[nb][:],
                rep[g % 4][:],
                wT[:, g, nb * BLOCK:(nb + 1) * BLOCK],
                start=(g == 0),
                stop=(g == 7),
            )
    for nb in range(2):
        nc.vector.tensor_sub(
            b_rep[:, nb * BLOCK:(nb + 1) * BLOCK],
            b_rep[:, nb * BLOCK:(nb + 1) * BLOCK],
            bc_ps[nb][:],
        )

    wth.release()
    late.release()

    # ------------- pass D: out = x @ w^T + b' -----------------------------
    pd = tc.alloc_tile_pool(name="passD", bufs=6, side="right")
    pd_out = tc.alloc_tile_pool(name="passDout", bufs=3, side="right")

    n_steps = len(PASSD_STEPS)
    for st in range(n_steps):
        c, off, nt = PASSD_STEPS[st]
        xT16 = xT_pre[st] if st < PRE_T else issue_xT(st, pd)
        ot = pd_out.tile([PART, nt, D], f32, tag="ot", name="ot")
        for tt in range(nt):
            pts = [ps_ns.tile([PART, BLOCK], f32, tag="mm", name=f"outp{nb}")
                   for nb in range(2)]
            for g in range(8):
                for nb in range(2):
                    nc.tensor.matmul(
                        pts[nb][:],
                        xT16[:, tt * 8 + g, :],
                        wT[:, g, nb * BLOCK:(nb + 1) * BLOCK],
                        start=(g == 0),
                        stop=(g == 7),
                    )
            for nb in range(2):
                nc.vector.tensor_add(
                    ot[:, tt, nb * BLOCK:(nb + 1) * BLOCK], pts[nb][:],
                    b_rep[:, nb * BLOCK:(nb + 1) * BLOCK],
                )
        r0 = sum(CHUNK_SIZES[:c]) + off
        nc.sync.dma_start(
            out[r0 * PART:(r0 + nt) * PART, :].rearrange(
                "(t p) f -> p t f", p=PART
            ),
            ot[:],
        )

    ps_ns.release()
    pd_out.release()
    pd.release()
    pd0.release()
    pa.release()
    wts2.release()
    consts.release()
    persist.release()
    dram.release()


# ---------------------------------------------------------------------------
def make_aux_inputs():
    import ml_dtypes

    return {
        "eye15": (1.5 * np.eye(PART)).astype(ml_dtypes.bfloat16),
        "id_f32": np.eye(PART, dtype=np.float32),
    }


_NC_CACHE = {}


def get_nc():
    if "nc" not in _NC_CACHE:
        _NC_CACHE["nc"] = build_nc()
    return _NC_CACHE["nc"]


def make_in_maps(x, weight, bias):
    import ml_dtypes

    aux = make_aux_inputs()
    x_bf = np.ascontiguousarray(
        np.asarray(x, dtype=np.float32).astype(ml_dtypes.bfloat16)
    )
    weightT_bf = np.ascontiguousarray(
        np.asarray(weight, dtype=np.float32).T.astype(ml_dtypes.bfloat16)
    )
    bias = np.asarray(bias, dtype=np.float32)
    bias_rep = np.ascontiguousarray(np.tile(bias[None, :], (PART, 1)))
    rows_pc = N_ROW_TILES * PART
    in_maps = []
    for i in range(N_CORES):
        m = {"x_bf": x_bf[i * rows_pc:(i + 1) * rows_pc],
             "weightT_bf": weightT_bf, "bias_rep": bias_rep}
        m.update(aux)
        in_maps.append(m)
    return in_maps


def kernel(x, weight, bias):
    nc = get_nc()
    in_maps = make_in_maps(x, weight, bias)
    res = bass_utils.run_bass_kernel_spmd(
        nc, in_maps, core_ids=list(range(N_CORES))
    )
    return np.concatenate([r["out"] for r in res.results], axis=0)
